# revision 41
# baseline (speedup 1.0000x reference)
# Trainium2 Bass kernel for the ContextBlock problem.
#
# Reference computation (per sample b):
#   xc    = concat(x0..x3)            [C=1024, HW=4096]
#   attn  = softmax(wm @ xc)          [HW]
#   ctx   = xc @ attn                 [C]
#   mul   = residual-gated MLP stack (sigmoid branch)   [C]
#   add   = residual-gated MLP stack (linear branch)    [C]
#   out   = sum_l (x_l * mul_l + add_l)                 [CL=256, HW]
#
# Distribution: data-parallel over batch, one sample per NeuronCore (B=8).
# No collectives required.
#
# Measured engine realities this kernel is built around:
#   - PE streams ~1.0-1.2 G cols/s on HW (chip power cap with 8 busy
#     cores; the nominal 2.4 G never materializes). PE columns are the
#     scarce resource.
#   - Pool (GpSimd) has NO PSUM access and only tensor_tensor /
#     tensor_copy on SBUF (~2.4 us per [128,1024] op).
#   - Scalar (Act) does func(scale*in + bias) with per-partition AP
#     scale/bias + free-axis accum_out, reads PSUM, 1.2 GHz.
#   - DVE does STT/tensor_scalar, reads PSUM, ~1 elem/lane/cycle.
#   - Each dma_start costs ~0.7 us of issue time on its queue.
#
# Dataflow:
#   x arrives CHUNK-major (spatial 512-column chunks spanning all 1024
#   channels) so pass1 logits complete per-chunk. pass1 runs in 4 PSUM
#   groups of 2 chunks; each group's exp/attn-broadcast fires as soon as
#   the group stops, and pass2 (unnormalized sum of exp(l)*x, 1/Z folded
#   in at the end) pipelines behind it on DVE (slabs 0-5) and Pool+
#   Scalar (slabs 6-7: TT product + accum-copy). v0 completes slab by
#   slab, and round-0 W1 matmuls chase those completions on the PE.
#   Gates: branch-streamed W1/W2 v-stationary matmuls, rows->columns
#   via 1-col matmuls, batched LayerNorm on DVE (Ln/Exp/Relu on Scalar),
#   sigmoid = 1/(1+exp(-z)).
#   pass3: out rows 0:128 on the PE (diag(mul) matmuls, bias folded into
#   drains), rows 128:256 as x*m+acc chains on Scalar/DVE merged by
#   Pool; per-chunk DMA-out on the sync queue.

import numpy as np
import ml_dtypes
from contextlib import ExitStack

import concourse.bass as bass
import concourse.bacc as bacc
import concourse.mybir as mybir
import concourse.tile as tile

BF = mybir.dt.bfloat16
F32 = mybir.dt.float32
AF = mybir.ActivationFunctionType
ALU = mybir.AluOpType
AX = mybir.AxisListType

B, L, CL, H, W = 8, 4, 256, 64, 64
C = L * CL          # 1024
HW = H * W          # 4096
P = C // 4          # 256
R = 2
EPS = 1e-5
NJ = C // 128       # 8   c-slabs
NG = 2              # pass1/pass2 spatial groups (halves)
NCORES = 8

_CACHE = {}


def _patch_act_tables():
    """Prune our activation functions from every table set except
    natural_log_exp_and_others so the whole kernel runs on ONE set."""
    if getattr(bacc, "_act_tables_patched", False):
        return
    from concourse import hw_specs
    orig = hw_specs.get_activation_tables
    mine = {AF.Exp, AF.Ln, AF.Relu, AF.Identity, AF.Copy}
    keep = "natural_log_exp_and_others"

    def patched(arch):
        tabs = orig(arch)
        out = {}
        for name, fns in tabs.items():
            out[name] = set(fns) if name == keep else set(fns) - mine
        return out

    import functools
    patched_cached = functools.cache(patched)
    bacc.get_activation_tables = patched_cached
    bacc._act_tables_patched = True


def _build_nc():
    _patch_act_tables()
    nc = bacc.Bacc()

    x_d = nc.dram_tensor("x", [C, HW], BF, kind="ExternalInput")
    wmc_d = nc.dram_tensor("wmc", [128, 16, 2], BF, kind="ExternalInput")
    bfc_d = nc.dram_tensor("bfc", [128, 800], BF, kind="ExternalInput")
    sm_d = nc.dram_tensor("smalls", [128, 272], F32, kind="ExternalInput")
    wg1_d = nc.dram_tensor("wg1", [2, 8, 128, 2048], BF, kind="ExternalInput")
    wg2_d = nc.dram_tensor("wg2", [2, 128, 4096], BF, kind="ExternalInput")
    out_d = nc.dram_tensor("out", [CL, HW], BF, kind="ExternalOutput")

    with tile.TileContext(nc) as tc, ExitStack() as ctx:
        resid = ctx.enter_context(tc.tile_pool(name="resid", bufs=1))
        spool = ctx.enter_context(tc.tile_pool(name="spool", bufs=1))

        x_sb = resid.tile([128, NJ, HW], BF, tag="x")
        wmc = resid.tile([128, 16, 2], BF, tag="wmc")
        bfc = resid.tile([128, 800], BF, tag="bfc")
        sm = resid.tile([128, 272], F32, tag="sm")
        wg1_sb = resid.tile([128, 2, 8, 2048], BF, tag="wg1")
        wg2_sb = resid.tile([128, 2, 4096], BF, tag="wg2")
        attn_bc = resid.tile([128, HW], BF, tag="attn_bc")
        scrD = resid.tile([128, 2048], BF, tag="scrD")
        xT2 = resid.tile([128, 2, 32, 128], BF, tag="xT2")

        # ---- DMA issue order: wmc, x h0 slabs, consts, x h1 slabs,
        # then gate weights (both rounds; they stream behind x).
        nc.sync.dma_start(wmc[:], wmc_d[:])
        nc.sync.dma_start(x_sb[:, 0, 0:1024], x_d[0:128, 0:1024])
        nc.sync.dma_start(x_sb[:, 0, 1024:2048], x_d[0:128, 1024:2048])
        for j in range(1, NJ):
            nc.sync.dma_start(x_sb[:, j, 0:2048],
                              x_d[128 * j:128 * (j + 1), 0:2048])
        nc.sync.dma_start(bfc[:], bfc_d[:])
        nc.sync.dma_start(sm[:], sm_d[:])
        for j in range(NJ):
            nc.sync.dma_start(x_sb[:, j, 2048:HW],
                              x_d[128 * j:128 * (j + 1), 2048:HW])
        for r in range(R):
            for j in range(NJ):
                nc.sync.dma_start(wg1_sb[:, r, j, :], wg1_d[r, j])
            nc.sync.dma_start(wg2_sb[:, r], wg2_d[r])

        ident = bfc[:, 0:128]
        ident2 = bfc[0:2, 0:2]
        ones_col_bf = bfc[:, 128:129]
        ones2_bf = bfc[0:2, 128:129]
        one0 = bfc[0:1, 128:129]

        def e2(cc):   # [2, 128] selector: row cc = ones
            return bfc[0:2, 288 + 128 * cc:288 + 128 * (cc + 1)]

        onesf = sm[0:1, 128:256]
        cm256 = sm[0:1, 257:258]   # -1/256 (LN variance fold)

        def b2c(r, b):
            return sm[:, 96 + 16 * r + 8 * b:96 + 16 * r + 8 * b + 8]

        # =============== phase A: pass1 + pass2, group-pipelined =====
        acc2 = spool.tile([2, 2 * NG], F32, tag="acc2")
        inv = spool.tile([1, 1], F32, tag="inv")
        inv_bc = spool.tile([128, 1], F32, tag="inv_bc")
        v0 = spool.tile([128, NJ], F32, tag="v0")
        v0g = spool.tile([128, NJ], BF, tag="v0g")
        v0p = spool.tile([128, NJ, NG], F32, tag="v0p")
        attnG = [[spool.tile([2, 512], BF, tag=f"attnG{g}{ab}",
                              name=f"attnG{g}{ab}") for ab in range(2)]
                 for g in range(NG)]

        with tc.tile_pool(name="psA", bufs=1,
                          space=bass.MemorySpace.PSUM) as psA:
            lgG = [[psA.tile([2, 512], F32, tag=f"lg{g}{ab}",
                              name=f"lg{g}{ab}") for ab in range(2)]
                   for g in range(NG)]

            # PE: pass1 matmuls, group-major (G = spatial half), riding
            # the half-slab DMAs with TWO interleaved accumulation
            # chains per group (independent PSUM banks pipeline ~2x
            # better than one chain); then exp + attn broadcast, then
            # pass2 STT on DVE.
            for g in range(NG):
                for j in range(NJ):
                    for cp in range(2):
                        for ab in range(2):
                            c = 4 * g + 2 * ab + cp
                            nc.tensor.matmul(
                                lgG[g][ab][:],
                                wmc[:, 2 * j + cp, :],
                                x_sb[:, j, 512 * c:512 * (c + 1)],
                                start=(cp == 0 and j == 0),
                                stop=(cp == 1 and j == NJ - 1),
                            )
                # |logits| < ~4: exp without max subtraction; Z via accum
                for ab in range(2):
                    nc.scalar.activation(attnG[g][ab][:], lgG[g][ab][:],
                                         AF.Exp,
                                         accum_out=acc2[:, 2 * g + ab:
                                                        2 * g + ab + 1])
                # unnormalized attn broadcast for this group
                for ab in range(2):
                    for cp in range(2):
                        c = 4 * g + 2 * ab + cp
                        pb = psA.tile([128, 512], F32, tag="bcb",
                                      name=f"bc{c % 2}")
                        nc.tensor.matmul(pb[:], e2(cp), attnG[g][ab][:])
                        nc.scalar.copy(attn_bc[:, 512 * c:512 * (c + 1)],
                                       pb[:])

                if g == NG - 1:
                    # 1/Z (both groups' accumulators are in)
                    accZ = spool.tile([2, 1], F32, tag="accZ")
                    nc.vector.reduce_sum(out=accZ[:], in_=acc2[:],
                                         axis=AX.X)
                    accZb = spool.tile([2, 1], BF, tag="accZb")
                    nc.vector.tensor_copy(accZb[:], accZ[:])
                    ps_s = psA.tile([1, 1], F32, tag="small", bufs=1)
                    nc.tensor.matmul(ps_s[:], accZb[:], ones2_bf)
                    nc.vector.reciprocal(inv[:], ps_s[:])
                    ps_ib = psA.tile([128, 1], F32, tag="small", bufs=1)
                    nc.tensor.matmul(ps_ib[:], onesf, inv[:])
                    nc.vector.tensor_copy(inv_bc[:], ps_ib[:])

                # pass2 for this group: DVE STT with free-axis accum,
                # slab-major so v0 completes slab by slab in group 1
                gs = slice(2048 * g, 2048 * (g + 1))
                for j in range(6):
                    nc.vector.scalar_tensor_tensor(
                        out=scrD[:], in0=x_sb[:, j, gs], scalar=1.0,
                        in1=attn_bc[:, gs], op0=ALU.mult, op1=ALU.mult,
                        accum_out=v0p[:, j, g:g + 1],
                    )
                    if g == NG - 1:
                        # v0 for this slab is complete: finalize it so
                        # the round-0 W1 matmuls can chase
                        nc.vector.reduce_sum(out=v0[:, j:j + 1],
                                             in_=v0p[:, j, :], axis=AX.X)
                        nc.vector.tensor_scalar_mul(v0g[:, j:j + 1],
                                                    v0[:, j:j + 1],
                                                    inv_bc[:])

            # T-route for slabs 6-7: PE transposes + attn-column ctx
            for jj in range(2):
                for t in range(4):
                    px = psA.tile([128, 1024], BF, tag=f"lg0{t % 2}",
                                  name=f"lg0{t % 2}")
                    for u in range(8):
                        g8 = 8 * t + u
                        nc.tensor.transpose(
                            px[:, 128 * u:128 * (u + 1)],
                            x_sb[:, 6 + jj, 128 * g8:128 * (g8 + 1)],
                            ident,
                        )
                    nc.scalar.copy(
                        xT2[:, jj, 8 * t:8 * (t + 1), :],
                        px[:].rearrange("p (u c) -> p u c", c=128),
                    )
            # attn columns: attnT col = 16g + 8ab + 2kk + cp
            psAT = psA.tile([128, 2, 2, 4, 2], BF, tag="lg10", name="lg10")
            for g in range(NG):
                for ab in range(2):
                    for kk in range(4):
                        nc.tensor.transpose(
                            psAT[:, g, ab, kk, :],
                            attnG[g][ab][0:2, 128 * kk:128 * (kk + 1)],
                            ident2,
                        )
            attnT = spool.tile([128, 32], BF, tag="attnT")
            nc.scalar.copy(attnT[:], psAT[:].rearrange(
                "p a b c d -> p (a b c d)"))
            ctx67 = psA.tile([1, 256], F32, tag="lg11", name="lg11")
            for m in range(32):
                g, rem = divmod(m, 16)
                ab, rem2 = divmod(rem, 8)
                cp, kk = divmod(rem2, 4)
                col = 16 * g + 8 * ab + 2 * kk + cp
                nc.tensor.matmul(
                    ctx67[:], attnT[:, col:col + 1],
                    xT2[:, :, m, :], start=(m == 0), stop=(m == 31),
                )
            v0row67 = spool.tile([1, 256], BF, tag="v0row67")
            nc.vector.tensor_copy(v0row67[:], ctx67[:])
            psV67 = psA.tile([128, 2], F32, tag="small", bufs=1)
            for q in range(2):
                nc.tensor.matmul(psV67[:, q:q + 1],
                                 v0row67[0:1, 128 * q:128 * (q + 1)], one0)
            nc.vector.tensor_scalar_mul(v0g[:, 6:8], psV67[:], inv_bc[:])

        # =============== gates ===============
        gates_ctx = tc.tile_pool(name="psg", bufs=1,
                                 space=bass.MemorySpace.PSUM)
        psg = gates_ctx.__enter__()

        vmuls = []
        vadds = []
        vm1 = spool.tile([128, NJ], BF, tag="vm1")
        va1 = spool.tile([128, NJ], BF, tag="va1")

        def gate_round(r):
            tag = f"r{r}"

            def stat(b, j):
                if r == 0:
                    return v0g[:, j:j + 1]
                return (vm1 if b == 0 else va1)[:, j:j + 1]

            # W1: mul branch stream, then add branch stream
            psW = [psg.tile([1, 512], F32, tag="w1p", name=f"w1p{k}", bufs=4)
                   for k in range(4)]
            hrows = {}
            for b in range(2):
                for j in range(NJ):
                    for p in range(2):
                        nc.tensor.matmul(
                            psW[2 * b + p][:], stat(b, j),
                            wg1_sb[:, r, j,
                                   1024 * b + 512 * p:1024 * b + 512 * (p + 1)],
                            start=(j == 0), stop=(j == NJ - 1),
                        )
                hrow = spool.tile([1, 1024], BF, tag="rowbuf",
                                  name=f"hrow{tag}{b}")
                nc.scalar.copy(hrow[0:1, 0:512], psW[2 * b][:])
                nc.vector.tensor_copy(hrow[0:1, 512:1024], psW[2 * b + 1][:])
                hrows[b] = hrow

            # h rows -> columns, (b, l, t) layout: col 8b + k
            psT = psg.tile([128, 16], F32, tag="tp", name=f"tp{tag}", bufs=2)
            for b in range(2):
                for k in range(8):
                    nc.tensor.matmul(
                        psT[:, 8 * b + k:8 * b + k + 1],
                        hrows[b][0:1, 128 * k:128 * (k + 1)],
                        one0,
                    )

            # LayerNorm, both branches batched on DVE (Ln/Exp/Relu on
            # Scalar).  g pre-scaled by sqrt(P); invsigma_noP =
            # exp(-0.5*ln(S2 - S1^2/P + P*EPS)); mu folded as S1/P.
            hn_g = spool.tile([128, 16], BF, tag=f"hnbf{tag}")
            ps_st = psg.tile([1, 32], F32, tag="tiny", bufs=2)
            ps_bc2 = psg.tile([128, 32], F32, tag="tp", name=f"tpb{tag}",
                              bufs=2)

            stcat = spool.tile([128, 32], BF, tag="stcat")
            nc.vector.tensor_add(stcat[:, 0:16], psT[:],
                                 sm[:, 16 * r:16 * r + 16])
            nc.vector.tensor_mul(stcat[:, 16:32], stcat[:, 0:16],
                                 stcat[:, 0:16])
            nc.tensor.matmul(ps_st[:], ones_col_bf, stcat[:])

            w8 = spool.tile([1, 16], F32, tag="w8")
            nc.vector.reduce_sum(
                out=w8[0:1, 0:16],
                in_=ps_st[0:1, 0:32].rearrange("p (g t) -> p g t", t=2),
                axis=AX.X,
            )
            sq = spool.tile([1, 16], F32, tag="sq")
            nc.vector.tensor_mul(sq[0:1, 0:8], w8[0:1, 0:8], w8[0:1, 0:8])
            nc.vector.scalar_tensor_tensor(
                out=sq[0:1, 8:16], in0=sq[0:1, 0:8], scalar=cm256,
                in1=w8[0:1, 8:16], op0=ALU.mult, op1=ALU.add,
            )
            nc.vector.tensor_scalar_add(sq[0:1, 8:16], sq[0:1, 8:16],
                                        P * EPS)
            lnv = spool.tile([1, 16], F32, tag="lnv")
            nc.scalar.activation(lnv[0:1, 0:8], sq[0:1, 8:16], AF.Ln)
            nc.scalar.activation(lnv[0:1, 8:16], lnv[0:1, 0:8], AF.Exp,
                                 scale=-0.5)

            brow = spool.tile([1, 32], F32, tag="brow")
            bview = brow[0:1, 0:16].rearrange("p (g t) -> p t g", t=2)
            iview = brow[0:1, 16:32].rearrange("p (g t) -> p t g", t=2)
            for t in range(2):
                nc.vector.tensor_scalar_mul(bview[:, t, :], w8[0:1, 0:8],
                                            1.0 / P)
                nc.vector.tensor_copy(iview[:, t, :], lnv[0:1, 8:16])
            nc.tensor.matmul(ps_bc2[:], onesf, brow[:])

            hn = spool.tile([128, 16], F32, tag="hn")
            nc.vector.tensor_sub(hn[:], stcat[:, 0:16], ps_bc2[:, 0:16])
            nc.vector.tensor_mul(hn[:], hn[:], ps_bc2[:, 16:32])
            nc.vector.tensor_mul(hn[:], hn[:],
                                 sm[:, 32 + 16 * r:48 + 16 * r])
            nc.vector.tensor_add(hn[:], hn[:],
                                 sm[:, 64 + 16 * r:80 + 16 * r])
            nc.scalar.activation(hn_g[:], hn[:], AF.Relu)

            # W2: mul branch then add branch; z rows in psum (row 0)
            psZ = [psg.tile([1, 512], F32, tag="w1p", name=f"w1p{k}", bufs=4)
                   for k in range(4)]
            zrows = {}
            for b in range(2):
                # lv order alternates the two PSUM banks (chained
                # same-bank matmuls serialize ~1.05 G cols/s vs ~1.7 G
                # interleaved); per-region t accumulation order is kept
                for lv0 in range(2):
                    for t in range(2):
                        for lv in (lv0, lv0 + 2):
                            nc.tensor.matmul(
                                psZ[2 * b + lv // 2][
                                    0:1,
                                    256 * (lv % 2):256 * (lv % 2) + 256],
                                hn_g[:, 8 * b + 2 * lv + t:
                                     8 * b + 2 * lv + t + 1],
                                wg2_sb[:, r, 1024 * lv + 512 * t + 256 * b:
                                       1024 * lv + 512 * t + 256 * b + 256],
                                start=(t == 0), stop=(t == 1),
                            )
                zrow = spool.tile([1, 1024], BF, tag="rowbuf",
                                  name=f"zrow{tag}{b}")
                nc.scalar.copy(zrow[0:1, 0:512], psZ[2 * b][:])
                nc.vector.tensor_copy(zrow[0:1, 512:1024], psZ[2 * b + 1][:])
                zrows[b] = zrow

            psZT = psg.tile([128, 16], F32, tag="tp", name=f"tpz{tag}",
                            bufs=2)
            for b in range(2):
                for k in range(8):
                    nc.tensor.matmul(
                        psZT[:, 8 * b + k:8 * b + k + 1],
                        zrows[b][0:1, 128 * k:128 * (k + 1)],
                        one0,
                    )

            # z columns + b2; sigmoid(zm) = 1/(1+exp(-zm))
            vmul = spool.tile([128, 8], F32, tag=f"vm{tag}")
            vadd = spool.tile([128, 8], F32, tag=f"va{tag}")
            zcm = spool.tile([128, 8], F32, tag="zcm")
            e = spool.tile([128, 8], F32, tag="sge")
            nc.vector.tensor_add(zcm[:], psZT[:, 0:8], b2c(r, 0))
            nc.scalar.activation(e[:], zcm[:], AF.Exp, scale=-1.0)
            nc.vector.tensor_scalar_add(e[:], e[:], 1.0)
            nc.vector.reciprocal(vmul[:], e[:])
            nc.vector.tensor_add(vadd[:], psZT[:, 8:16], b2c(r, 1))
            vmuls.append(vmul)
            vadds.append(vadd)
            if r == 0:
                nc.vector.tensor_copy(vm1[:], vmul[:])
                nc.gpsimd.tensor_copy(va1[:], vadd[:])

        gate_round(0)
        gate_round(1)

        mm_f = spool.tile([128, NJ], F32, tag="mmf")
        nc.vector.tensor_add(mm_f[:], vmuls[0][:], vmuls[1][:])
        ma_f = spool.tile([128, NJ], F32, tag="maf")
        nc.gpsimd.tensor_add(ma_f[:], vadds[0][:], vadds[1][:])
        gates_ctx.__exit__(None, None, None)

        # =============== pass 3: output ===============
        late_ctx = tc.tile_pool(name="late", bufs=1)
        late = late_ctx.__enter__()

        addsum = spool.tile([128, 2], F32, tag="addsum")
        nc.vector.reduce_sum(
            out=addsum[:],
            in_=ma_f[:].rearrange("p (l t) -> p t l", t=2),
            axis=AX.X,
        )
        diags = []
        for js in range(8):
            dt_ = late.tile([128, 128], BF, tag=f"diag{js}", name=f"diag{js}")
            nc.vector.tensor_scalar_mul(dt_[:], ident,
                                        mm_f[:, js:js + 1])
            diags.append(dt_)

        with tc.tile_pool(name="ps3", bufs=1,
                          space=bass.MemorySpace.PSUM) as ps3:
            # PE does both output halves: jj half, l-outer accumulation
            # into 8 chunk banks, then drain while the other half runs.
            stg = [late.tile([128, 512], BF, tag=f"stg{k}", name=f"stg{k}")
                   for k in range(4)]
            for jj in range(2):
                chunks = [ps3.tile([128, 512], F32, tag=f"big{n}",
                                   name=f"big{n}") for n in range(8)]
                for lv in range(4):
                    for n in range(8):
                        nc.tensor.matmul(
                            chunks[n][:],
                            diags[2 * lv + jj][:],
                            x_sb[:, 2 * lv + jj, 512 * n:512 * (n + 1)],
                            start=(lv == 0), stop=(lv == 3),
                        )
                for n in range(8):
                    s = stg[(n % 2) + 2 * jj]
                    if n % 2 == 0:
                        nc.scalar.activation(
                            s[:], chunks[n][:], AF.Identity,
                            bias=addsum[:, jj:jj + 1], scale=1.0,
                        )
                    else:
                        nc.vector.tensor_scalar_add(s[:], chunks[n][:],
                                                    addsum[:, jj:jj + 1])
                    nc.sync.dma_start(
                        out_d[128 * jj:128 * (jj + 1),
                              512 * n:512 * (n + 1)], s[:],
                    )
        late_ctx.__exit__(None, None, None)

    nc.compile()
    return nc


def _pack_inputs(x0, x1, x2, x3, wm, bm,
                 add_W1, add_b1, add_g, add_be, add_W2, add_b2,
                 mul_W1, mul_b1, mul_g, mul_be, mul_W2, mul_b2):
    bf = ml_dtypes.bfloat16
    f32 = np.float32

    # pass1 stationaries: wmc[:, 2j + cp, cp] = wm slab j
    wmr = np.asarray(wm, f32).reshape(NJ, 128)
    wmc = np.zeros((128, 16, 2), f32)
    for cp in range(2):
        for j in range(NJ):
            wmc[:, 2 * j + cp, cp] = wmr[j]
    wmc = wmc.astype(bf)

    # bf16 const block: identity | ones col | pad | e2 selectors
    bfc = np.zeros((128, 800), f32)
    bfc[:, 0:128] = np.eye(128)
    bfc[:, 128] = 1.0
    for cc in range(2):
        bfc[cc, 288 + 128 * cc:288 + 128 * (cc + 1)] = 1.0
    bfc = bfc.astype(bf)

    W1s = [[mul_W1[r], add_W1[r]] for r in range(R)]
    W2s = [[mul_W2[r], add_W2[r]] for r in range(R)]
    b1s = [[mul_b1[r], add_b1[r]] for r in range(R)]
    gs = [[mul_g[r], add_g[r]] for r in range(R)]
    bes = [[mul_be[r], add_be[r]] for r in range(R)]
    b2s = [[mul_b2[r], add_b2[r]] for r in range(R)]

    sm = np.zeros((128, 272), f32)
    sm[:, 128:256] = 1.0
    sm[:, 256] = 1.0 / 256.0
    sm[:, 257] = -1.0 / 256.0

    def colmajor(v):  # [4,256]-like -> [128, 8] cols (l, t)
        return np.asarray(v, f32).reshape(4, 2, 128).transpose(2, 0, 1).reshape(128, 8)

    wg1 = np.zeros((2, 8, 128, 2048), f32)
    wg2 = np.zeros((2, 128, 4096), f32)

    for r in range(R):
        w2arr = np.zeros((128, 4, 2, 2, 256), f32)   # [pp, l, t, b, cl]
        for b in range(2):
            w1 = np.asarray(W1s[r][b], f32).reshape(C, C)  # [lp, c]
            t = w1.reshape(C, NJ, 128)                   # [q, j, cp]
            t = t.transpose(1, 2, 0)                     # [j, cp, q]
            wg1[r, :, :, 1024 * b:1024 * (b + 1)] = t
            w2 = np.asarray(W2s[r][b], f32)              # [l, cl, pp]
            t2 = w2.reshape(4, 256, 2, 128)              # [l, cl, tt, pp]
            t2 = t2.transpose(3, 0, 2, 1)                # [pp, l, tt, cl]
            w2arr[:, :, :, b, :] = t2
            sm[:, 16 * r + 8 * b:16 * r + 8 * b + 8] = colmajor(b1s[r][b])
            sm[:, 32 + 16 * r + 8 * b:32 + 16 * r + 8 * b + 8] = \
                colmajor(gs[r][b]) * float(np.sqrt(P))
            sm[:, 64 + 16 * r + 8 * b:64 + 16 * r + 8 * b + 8] = \
                colmajor(bes[r][b])
            sm[:, 96 + 16 * r + 8 * b:96 + 16 * r + 8 * b + 8] = \
                colmajor(b2s[r][b])
        wg2[r] = w2arr.reshape(128, 4096)

    shared = dict(wmc=wmc, bfc=bfc, smalls=sm,
                  wg1=wg1.astype(bf), wg2=wg2.astype(bf))

    in_maps = []
    xs = [np.asarray(a, f32) for a in (x0, x1, x2, x3)]
    for b in range(B):
        xc = np.concatenate(
            [a[b].reshape(CL, HW) for a in xs], axis=0
        ).astype(bf)
        in_maps.append({"x": xc, **shared})
    return in_maps


def kernel(**inputs):
    from concourse.bass_utils import run_bass_kernel_spmd

    if "nc" not in _CACHE:
        _CACHE["nc"] = _build_nc()
    nc = _CACHE["nc"]

    in_maps = _pack_inputs(**inputs)
    res = run_bass_kernel_spmd(nc, in_maps, list(range(NCORES)))
    _CACHE["last_results"] = res
    out = np.stack(
        [res.results[b]["out"].reshape(CL, H, W) for b in range(B)]
    ).astype(np.float32)
    return out
